# revision 22
# baseline (speedup 1.0000x reference)
"""fp8-DoubleRow contrastive-loss kernel for 8 NeuronCores.

s = xn @ xn.T is symmetric: only the 136 unordered band pairs {a, b} of a
16x16 grid of 512x512 cells are computed, via a star decomposition that is
CORE-UNIFORM in slot space: core c stores band (c+s) mod 16 in SBUF slot s
and runs two "stars":
  star 0 (a-slot 0): diag cell (0,0) + off-diag cells (0, s) s=1..8
  star 1 (a-slot 8): diag cell (8,8) + off-diag cells (8, s) s=9..15
Every unordered pair {i, j} with circular distance d=(j-i) mod 16 in 1..8 is
covered exactly once (by v=i if d<=7 or d=8 with i<8), 17 cells per core.

All 16 bands live in SBUF as fp8 (8 KB/partition each, 128 KB total), loaded
once by DMA (~47 us serialized) and reused by every matmul; operands never
re-stream. Column labels are DMA'd as a [1, N] vector and broadcast to all
128 partitions on the otherwise-idle GPSIMD engine, keeping the DMA stream
bands-only (the stream rate is what gates the PE early on).

Math per off-diag cell, with xq = round_fp8(xn * 256):
  PE : G = xq_a @ xq_b.T accumulated over 8 DoubleRow k-pairs (K=2048)
  ACT: e = exp(G / (256^2 tau)) -> fp8, fused f32 row-sum accum
  DVE: em = (yc == yo) * e -> fp8, fused f32 row-sum accum
  PE : colsum_e = ones.T @ e, colsum_em = ones.T @ em (DoubleRow over rb
       pairs), both copied into one [1,1024] tile by DVE, one DMA per cell.
Diag cells use a bf16 e (exp(10) overflows fp8) and skip colsums.
Off-diag cells are processed in PAIRS sharing one [128,1024] PSUM tile and
one ACT/DVE op, halving per-op overheads. Colsum matmuls for group g are
emitted after the first main-matmul block of group g+1 so the PE stream
never waits on ACT/DVE results (keeps the tensor-engine p-state at full
clock). The schedule ends on a diag cell: no colsum work in the tail.

Host: normalize + quantize (renormalized in the quantized domain) + band
layout; final assembly sums row/col partials per band in f64 and takes
mean(log(sum_all) - log(sum_same)).
"""

import sys

import numpy as np
import ml_dtypes

if "/opt/trn_rl_repo" not in sys.path:
    sys.path.insert(0, "/opt/trn_rl_repo")

import concourse.bass as bass  # noqa: F401  (bass types via tile/bacc)
import concourse.tile as tile
from concourse import bacc, mybir
from concourse.bass_utils import run_bass_kernel_spmd

TAU = 0.1
N, D = 8192, 2048
NCORES = 8
NB = 16                    # 512-row bands
BS = N // NB               # 512
KC = D // 128              # 16 contraction chunks
KP = KC // 2               # 8 DoubleRow k-pairs
RBC = BS // 128            # 4 row-blocks per band
QS = 256.0                 # fp8 quantization scale on xn
ASCALE = 1.0 / (QS * QS * TAU)
BF16 = mybir.dt.bfloat16
F32 = mybir.dt.float32
FP8 = mybir.dt.float8e4
DR = mybir.MatmulPerfMode.DoubleRow
NP_FP8 = ml_dtypes.float8_e4m3
NP_BF16 = ml_dtypes.bfloat16

# (a_slot, kind, b_slots); two stars, cells paired so ACT/DVE work on
# [128, 1024] tiles where possible. The single sits mid-schedule and the
# final group is a diag (no colsum chain in the tail).
GROUPS = [
    (0, "diag", (0,)),
    (0, "pair", (1, 2)),
    (0, "pair", (3, 4)),
    (0, "pair", (5, 6)),
    (0, "pair", (7, 8)),
    (8, "single", (15,)),
    (8, "pair", (9, 10)),
    (8, "pair", (11, 12)),
    (8, "pair", (13, 14)),
    (8, "diag", (8,)),
]
NG = len(GROUPS)           # 10 -> 40 row-accum slots (x2 for all/same)
NOFF = 15                  # off-diag cells; cols slot for b_slot s is s-1

# DMA stream order (single serialized DMA device in the model): bands as
# early as possible — they gate the PE. Early bands are split (band0 in
# quarters, bands 1-2 in halves) so the PE can start on the first kc-pairs
# while the rest streams in; later bands are single DMAs to minimize
# per-DMA overhead. Labels ride as one tiny [1, N] vector up front.
DMA_PLAN = {
    0: [("bandq", 0, 4), ("ycv",), ("yo",), ("band", 1), ("band", 2)],
    1: [("band", 3), ("band", 4)],
    2: [("band", 5), ("band", 6)],
    3: [("band", 7), ("band", 8)],
    4: [("band", 15)],
    5: [("band", 9), ("band", 10)],
    6: [("band", 11), ("band", 12), ("rows", 0)],
    7: [("band", 13), ("band", 14)],
}


def build_bass():
    nc = bacc.Bacc(None, target_bir_lowering=False)

    bands_d = nc.dram_tensor("bands", [NB * 128, KC, BS], FP8, kind="ExternalInput")
    yc_d = nc.dram_tensor("yc", [1, N], BF16, kind="ExternalInput")
    yo_d = nc.dram_tensor("yo", [128, 8], BF16, kind="ExternalInput")
    # rows: per (group, rb) slot, [all_h0, all_h1, same] -> [128, 120].
    # Pair groups split the exp over the two PSUM banks (two ACT ops) so the
    # ACT cadence stays under the PE cadence; their row sums land in two
    # accum slots that the host adds.
    rows_d = nc.dram_tensor("rows", [128, NG * RBC * 3], F32, kind="ExternalOutput")
    # cols: per off-diag cell, [all(512) | same(512)] -> [1, 15*1024]
    cols_d = nc.dram_tensor("cols", [1, NOFF * 1024], F32, kind="ExternalOutput")

    with (
        tile.TileContext(nc) as tc,
        tc.tile_pool(name="res", bufs=1) as res,
        tc.tile_pool(name="ep", bufs=2) as ep,
        tc.tile_pool(name="emp", bufs=2) as emp,
        tc.tile_pool(name="dgp", bufs=2) as dgp,
        tc.tile_pool(name="csp", bufs=4) as csp,
        tc.tile_pool(name="pp", bufs=2, space="PSUM") as pp,
        tc.tile_pool(name="cpp", bufs=4, space="PSUM") as cpp,
    ):
        band_t = [res.tile([128, KC, BS], FP8, name=f"band{s}") for s in range(NB)]
        ycv_t = res.tile([1, N], BF16)
        yc_t = res.tile([128, N], BF16)
        yo_t = res.tile([128, 8], BF16)
        # DoubleRow weights need a 3D AP [K, 2, M] with pair-step % 16 B == 0.
        ones_t = res.tile([128, 2, 16], FP8)
        nc.vector.memset(ones_t[:], 1.0)
        stage = res.tile([128, NG * RBC * 3], F32)
        nc.vector.memset(stage[:], 0.0)

        def emit_dma(item):
            if item[0] == "band":
                s = item[1]
                nc.sync.dma_start(
                    out=band_t[s][:], in_=bands_d[s * 128 : (s + 1) * 128, :, :]
                )
            elif item[0] == "bandq":
                # kc-splits: the first DoubleRow matmuls can start as soon as
                # the first chunk lands (subtile deps).
                s, nsp = item[1], item[2]
                step = KC // nsp
                for hh in range(nsp):
                    nc.sync.dma_start(
                        out=band_t[s][:, hh * step : (hh + 1) * step, :],
                        in_=bands_d[s * 128 : (s + 1) * 128, hh * step : (hh + 1) * step, :],
                    )
            elif item[0] == "yo":
                nc.gpsimd.dma_start(out=yo_t[:], in_=yo_d[:])
            elif item[0] == "rows":
                hh = item[1]
                half = NG * RBC * 3 // 2  # 60 of 120 columns
                sl = slice(hh * half, (hh + 1) * half)
                nc.sync.dma_start(out=rows_d[:, sl], in_=stage[:, sl])
            else:  # ycv: tiny label vector; broadcast on idle GPSIMD
                nc.gpsimd.dma_start(out=ycv_t[:], in_=yc_d[:])
                for q in range(4):
                    sl = slice(q * 4 * BS, (q + 1) * 4 * BS)
                    nc.gpsimd.partition_broadcast(yc_t[:, sl], ycv_t[:, sl])

        pending_cols = None
        col_dmas: list = []
        for g, (a_slot, kind, b_slots) in enumerate(GROUPS):
            for item in DMA_PLAN.get(g, ()):
                emit_dma(item)
            for cidx, cs in col_dmas:
                nc.sync.dma_start(
                    out=cols_d[:, cidx * 1024 : (cidx + 1) * 1024], in_=cs[:]
                )
            col_dmas.clear()

            diag = kind == "diag"
            w = len(b_slots) * BS
            if not diag:
                e_t = ep.tile([128, RBC, 1024], FP8, name="e_t")
                em_t = emp.tile([128, RBC, 1024], FP8, name="em_t")
            for rb in range(RBC):
                ps = pp.tile([128, 1024], F32, name="ps")
                for h, b in enumerate(b_slots):
                    for t in range(KP):
                        nc.tensor.matmul(
                            ps[:, h * BS : (h + 1) * BS],
                            band_t[a_slot][:, 2 * t : 2 * t + 2, rb * 128 : (rb + 1) * 128],
                            band_t[b][:, 2 * t : 2 * t + 2, :],
                            start=(t == 0),
                            stop=(t == KP - 1),
                            perf_mode=DR,
                        )
                # Colsums of the PREVIOUS off-diag group: its e/em tiles are
                # ready by now, so the PE never waits on ACT/DVE, and the
                # copies/DMAs still clear well before the tail. For the final
                # (diag) group flush one rb later: its first exps then drain
                # during the flush, shortening the end-of-schedule ACT ladder.
                flush_rb = 1 if g == NG - 1 else 0
                if rb == flush_rb and pending_cols is not None:
                    pending_cols()
                    pending_cols = None
                slot = g * RBC + rb
                ycol = (0 if a_slot == 0 else 4) + rb
                yc_in = yc_t[:, b_slots[0] * BS : b_slots[0] * BS + w]
                if diag:
                    e_dg = dgp.tile([128, BS], BF16, name="e_dg")
                    nc.scalar.activation(
                        out=e_dg[:],
                        in_=ps[:, 0:BS],
                        func=mybir.ActivationFunctionType.Exp,
                        scale=ASCALE,
                        accum_out=stage[:, 3 * slot : 3 * slot + 1],
                    )
                    em_dg = dgp.tile([128, BS], BF16, name="em_dg")
                    nc.vector.scalar_tensor_tensor(
                        out=em_dg[:],
                        in0=yc_in,
                        scalar=yo_t[:, ycol : ycol + 1],
                        in1=e_dg[:],
                        op0=mybir.AluOpType.is_equal,
                        op1=mybir.AluOpType.mult,
                        accum_out=stage[:, 3 * slot + 2 : 3 * slot + 3],
                    )
                else:
                    nc.scalar.activation(
                        out=e_t[:, rb, 0:w],
                        in_=ps[:, 0:w],
                        func=mybir.ActivationFunctionType.Exp,
                        scale=ASCALE,
                        accum_out=stage[:, 3 * slot : 3 * slot + 1],
                    )
                    nc.vector.scalar_tensor_tensor(
                        out=em_t[:, rb, 0:w],
                        in0=yc_in,
                        scalar=yo_t[:, ycol : ycol + 1],
                        in1=e_t[:, rb, 0:w],
                        op0=mybir.AluOpType.is_equal,
                        op1=mybir.AluOpType.mult,
                        accum_out=stage[:, 3 * slot + 2 : 3 * slot + 3],
                    )

            if not diag:

                def make_cols(e_t=e_t, em_t=em_t, b_slots=b_slots, last=(g == NG - 2)):
                    def emit():
                        for h, b in enumerate(b_slots):
                            cs = csp.tile([1, 1024], F32, name="cs", tag="cs")
                            cps_e = cpp.tile([1, BS], F32, name="cps_e", tag="col")
                            for t2 in range(2):
                                nc.tensor.matmul(
                                    cps_e[:],
                                    ones_t[:, 0:2, 0:1],
                                    e_t[:, 2 * t2 : 2 * t2 + 2, h * BS : (h + 1) * BS],
                                    start=(t2 == 0),
                                    stop=(t2 == 1),
                                    perf_mode=DR,
                                )
                            cps_m = cpp.tile([1, BS], F32, name="cps_m", tag="col")
                            for t2 in range(2):
                                nc.tensor.matmul(
                                    cps_m[:],
                                    ones_t[:, 0:2, 0:1],
                                    em_t[:, 2 * t2 : 2 * t2 + 2, h * BS : (h + 1) * BS],
                                    start=(t2 == 0),
                                    stop=(t2 == 1),
                                    perf_mode=DR,
                                )
                            # Copies ride on the otherwise-idle GPSIMD so
                            # neither ACT (exp chain) nor DVE (mask chain)
                            # ever backs up behind them.
                            nc.gpsimd.tensor_copy(out=cs[:, 0:BS], in_=cps_e[:])
                            nc.gpsimd.tensor_copy(out=cs[:, BS:1024], in_=cps_m[:])
                            cidx = b - 1
                            col_dmas.append(
                                (cidx, cs)
                            )

                    return emit

                pending_cols = make_cols()

        if pending_cols is not None:
            pending_cols()
        # Tail: the bulk of rows h1 (groups 5..8) is ready before the final
        # diag finishes — issue it first so only the last 12 columns ride on
        # the final mask op. The last cols go out via the idle Pool queue so
        # the SP sequencer never blocks the rows chain.
        half = NG * RBC * 3 // 2
        tail = (NG - 1) * RBC * 3
        nc.sync.dma_start(out=rows_d[:, half:tail], in_=stage[:, half:tail])
        for cidx, cs in col_dmas:
            nc.gpsimd.dma_start(out=cols_d[:, cidx * 1024 : (cidx + 1) * 1024], in_=cs[:])
        col_dmas.clear()
        nc.sync.dma_start(out=rows_d[:, tail:], in_=stage[:, tail:])

    nc.compile()
    return nc


_CACHE: dict = {}


def _get_nc():
    if "nc" not in _CACHE:
        _CACHE["nc"] = build_bass()
    return _CACHE["nc"]


def _prep_inputs(x, y):
    x = np.ascontiguousarray(np.asarray(x, dtype=np.float32))
    y = np.asarray(y).astype(np.int64)
    xn = x / np.linalg.norm(x, axis=1, keepdims=True)
    # Renormalize in the quantized domain: rescale each row so its QUANTIZED
    # norm is exactly QS, killing the systematic s_ii bias from fp8 rounding.
    xq = (xn * QS).astype(NP_FP8)
    nrm = np.sqrt((xq.astype(np.float32) ** 2).sum(1, keepdims=True)) / QS
    xq = ((xn * QS) / nrm).astype(NP_FP8)
    ybf = y.astype(NP_BF16)

    # band[b][p, kc, jj] = xq[b*BS + jj, kc*128 + p]
    bandmat = [
        np.ascontiguousarray(xq[b * BS : (b + 1) * BS].reshape(BS, KC, 128).transpose(2, 1, 0))
        for b in range(NB)
    ]

    in_maps = []
    for c in range(NCORES):
        perm = [(c + s) % NB for s in range(NB)]
        bands = np.concatenate([bandmat[b] for b in perm], axis=0)
        yc = np.ascontiguousarray(
            np.concatenate([ybf[b * BS : (b + 1) * BS] for b in perm])
        ).reshape(1, N)
        yo = np.ascontiguousarray(
            np.concatenate(
                [
                    ybf[perm[a] * BS : (perm[a] + 1) * BS].reshape(RBC, 128).T
                    for a in (0, 8)
                ],
                axis=1,
            )
        )
        in_maps.append({"bands": bands, "yc": yc, "yo": yo})
    return in_maps


def _assemble(results):
    sum_all = np.zeros(N, dtype=np.float64)
    sum_same = np.zeros(N, dtype=np.float64)
    for c in range(NCORES):
        r = results[c]
        perm = [(c + s) % NB for s in range(NB)]
        rows = r["rows"].astype(np.float64)        # [128, 120]: [all_h0, all_h1, same]
        cols = r["cols"].astype(np.float64).reshape(NOFF, 2, BS)
        for g, (a_slot, kind, b_slots) in enumerate(GROUPS):
            ab = perm[a_slot]
            for rb in range(RBC):
                rr = ab * BS + rb * 128 + np.arange(128)
                slot = g * RBC + rb
                sum_all[rr] += rows[:, 3 * slot] + rows[:, 3 * slot + 1]
                sum_same[rr] += rows[:, 3 * slot + 2]
            if kind != "diag":
                for b in b_slots:
                    cc = perm[b] * BS + np.arange(BS)
                    cidx = b - 1
                    sum_all[cc] += cols[cidx, 0]
                    sum_same[cc] += cols[cidx, 1]
    loss = np.log(sum_all) - np.log(sum_same)
    return np.float32(loss.mean())


def run(x, y, trace=False, **spmd_kwargs):
    nc = _get_nc()
    in_maps = _prep_inputs(x, y)
    res = run_bass_kernel_spmd(
        nc, in_maps, core_ids=list(range(NCORES)), trace=trace, **spmd_kwargs
    )
    return _assemble(res.results), res


def kernel(x, y, fp_v=None, **_ignored):
    val, _ = run(x, y, trace=False)
    return np.asarray(val, dtype=np.float32)


# revision 23
# speedup vs baseline: 1.0097x; 1.0097x over previous
"""fp8-DoubleRow contrastive-loss kernel for 8 NeuronCores.

s = xn @ xn.T is symmetric: only the 136 unordered band pairs {a, b} of a
16x16 grid of 512x512 cells are computed, via a star decomposition that is
CORE-UNIFORM in slot space: core c stores band (c+s) mod 16 in SBUF slot s
and runs two "stars":
  star 0 (a-slot 0): diag cell (0,0) + off-diag cells (0, s) s=1..8
  star 1 (a-slot 8): diag cell (8,8) + off-diag cells (8, s) s=9..15
Every unordered pair {i, j} with circular distance d=(j-i) mod 16 in 1..8 is
covered exactly once (by v=i if d<=7 or d=8 with i<8), 17 cells per core.

All 16 bands live in SBUF as fp8 (8 KB/partition each, 128 KB total), loaded
once by DMA (~47 us serialized) and reused by every matmul; operands never
re-stream. Column labels are DMA'd as a [1, N] vector and broadcast to all
128 partitions on the otherwise-idle GPSIMD engine, keeping the DMA stream
bands-only (the stream rate is what gates the PE early on).

Math per off-diag cell, with xq = round_fp8(xn * 256):
  PE : G = xq_a @ xq_b.T accumulated over 8 DoubleRow k-pairs (K=2048)
  ACT: e = exp(G / (256^2 tau)) -> fp8, fused f32 row-sum accum
  DVE: em = (yc == yo) * e -> fp8, fused f32 row-sum accum
  PE : colsum_e = ones.T @ e, colsum_em = ones.T @ em (DoubleRow over rb
       pairs), both copied into one [1,1024] tile by DVE, one DMA per cell.
Diag cells use a bf16 e (exp(10) overflows fp8) and skip colsums.
Off-diag cells are processed in PAIRS sharing one [128,1024] PSUM tile and
one ACT/DVE op, halving per-op overheads. Colsum matmuls for group g are
emitted after the first main-matmul block of group g+1 so the PE stream
never waits on ACT/DVE results (keeps the tensor-engine p-state at full
clock). The schedule ends on a diag cell: no colsum work in the tail.

Host: normalize + quantize (renormalized in the quantized domain) + band
layout; final assembly sums row/col partials per band in f64 and takes
mean(log(sum_all) - log(sum_same)).
"""

import sys

import numpy as np
import ml_dtypes

if "/opt/trn_rl_repo" not in sys.path:
    sys.path.insert(0, "/opt/trn_rl_repo")

import concourse.bass as bass  # noqa: F401  (bass types via tile/bacc)
import concourse.tile as tile
from concourse import bacc, mybir
from concourse.bass_utils import run_bass_kernel_spmd

TAU = 0.1
N, D = 8192, 2048
NCORES = 8
NB = 16                    # 512-row bands
BS = N // NB               # 512
KC = D // 128              # 16 contraction chunks
KP = KC // 2               # 8 DoubleRow k-pairs
RBC = BS // 128            # 4 row-blocks per band
QS = 256.0                 # fp8 quantization scale on xn
ASCALE = 1.0 / (QS * QS * TAU)
BF16 = mybir.dt.bfloat16
F32 = mybir.dt.float32
FP8 = mybir.dt.float8e4
DR = mybir.MatmulPerfMode.DoubleRow
NP_FP8 = ml_dtypes.float8_e4m3
NP_BF16 = ml_dtypes.bfloat16

# (a_slot, kind, b_slots); two stars, cells paired so ACT/DVE work on
# [128, 1024] tiles where possible. The single sits mid-schedule and the
# final group is a diag (no colsum chain in the tail).
GROUPS = [
    (0, "diag", (0,)),
    (0, "pair", (1, 2)),
    (0, "pair", (3, 4)),
    (0, "pair", (5, 6)),
    (0, "pair", (7, 8)),
    (8, "single", (15,)),
    (8, "pair", (9, 10)),
    (8, "pair", (11, 12)),
    (8, "pair", (13, 14)),
    (8, "diag", (8,)),
]
NG = len(GROUPS)           # 10 -> 40 row-accum slots (x2 for all/same)
NOFF = 15                  # off-diag cells; cols slot for b_slot s is s-1

# DMA stream order (single serialized DMA device in the model): bands as
# early as possible — they gate the PE. Early bands are split (band0 in
# quarters, bands 1-2 in halves) so the PE can start on the first kc-pairs
# while the rest streams in; later bands are single DMAs to minimize
# per-DMA overhead. Labels ride as one tiny [1, N] vector up front.
DMA_PLAN = {
    0: [("bandq", 0, 4), ("ycv",), ("yo",), ("band", 1), ("band", 2)],
    1: [("band", 3), ("band", 4)],
    2: [("band", 5), ("band", 6)],
    3: [("band", 7), ("band", 8)],
    4: [("band", 15)],
    5: [("band", 9), ("band", 10)],
    6: [("band", 11), ("band", 12), ("rows", 0)],
    7: [("band", 13), ("band", 14)],
}


def build_bass():
    nc = bacc.Bacc(None, target_bir_lowering=False)

    bands_d = nc.dram_tensor("bands", [NB * 128, KC, BS], FP8, kind="ExternalInput")
    yc_d = nc.dram_tensor("yc", [1, N], BF16, kind="ExternalInput")
    yo_d = nc.dram_tensor("yo", [128, 8], BF16, kind="ExternalInput")
    # rows: per (group, rb) slot, [all_h0, all_h1, same] -> [128, 120].
    # Pair groups split the exp over the two PSUM banks (two ACT ops) so the
    # ACT cadence stays under the PE cadence; their row sums land in two
    # accum slots that the host adds.
    rows_d = nc.dram_tensor("rows", [128, NG * RBC * 3], F32, kind="ExternalOutput")
    # cols: per off-diag cell, [all(512) | same(512)] -> [1, 15*1024]
    cols_d = nc.dram_tensor("cols", [1, NOFF * 1024], F32, kind="ExternalOutput")

    with (
        tile.TileContext(nc) as tc,
        tc.tile_pool(name="res", bufs=1) as res,
        tc.tile_pool(name="ep", bufs=2) as ep,
        tc.tile_pool(name="emp", bufs=2) as emp,
        tc.tile_pool(name="dgp", bufs=2) as dgp,
        tc.tile_pool(name="csp", bufs=4) as csp,
        tc.tile_pool(name="pp", bufs=2, space="PSUM") as pp,
        tc.tile_pool(name="cpp", bufs=4, space="PSUM") as cpp,
    ):
        band_t = [res.tile([128, KC, BS], FP8, name=f"band{s}") for s in range(NB)]
        ycv_t = res.tile([1, N], BF16)
        yc_t = res.tile([128, N], BF16)
        yo_t = res.tile([128, 8], BF16)
        # DoubleRow weights need a 3D AP [K, 2, M] with pair-step % 16 B == 0.
        ones_t = res.tile([128, 2, 16], FP8)
        nc.vector.memset(ones_t[:], 1.0)
        stage = res.tile([128, NG * RBC * 3], F32)
        nc.vector.memset(stage[:], 0.0)

        def emit_dma(item):
            if item[0] == "band":
                s = item[1]
                nc.sync.dma_start(
                    out=band_t[s][:], in_=bands_d[s * 128 : (s + 1) * 128, :, :]
                )
            elif item[0] == "bandq":
                # kc-splits: the first DoubleRow matmuls can start as soon as
                # the first chunk lands (subtile deps).
                s, nsp = item[1], item[2]
                step = KC // nsp
                for hh in range(nsp):
                    nc.sync.dma_start(
                        out=band_t[s][:, hh * step : (hh + 1) * step, :],
                        in_=bands_d[s * 128 : (s + 1) * 128, hh * step : (hh + 1) * step, :],
                    )
            elif item[0] == "yo":
                nc.gpsimd.dma_start(out=yo_t[:], in_=yo_d[:])
            elif item[0] == "rows":
                hh = item[1]
                half = NG * RBC * 3 // 2  # 60 of 120 columns
                sl = slice(hh * half, (hh + 1) * half)
                nc.sync.dma_start(out=rows_d[:, sl], in_=stage[:, sl])
            else:  # ycv: tiny label vector; broadcast on idle GPSIMD
                nc.gpsimd.dma_start(out=ycv_t[:], in_=yc_d[:])
                for q in range(4):
                    sl = slice(q * 4 * BS, (q + 1) * 4 * BS)
                    nc.gpsimd.partition_broadcast(yc_t[:, sl], ycv_t[:, sl])

        pending_cols = None
        col_dmas: list = []
        for g, (a_slot, kind, b_slots) in enumerate(GROUPS):
            for item in DMA_PLAN.get(g, ()):
                emit_dma(item)
            for cidx, cs in col_dmas:
                nc.sync.dma_start(
                    out=cols_d[:, cidx * 1024 : (cidx + 1) * 1024], in_=cs[:]
                )
            col_dmas.clear()

            diag = kind == "diag"
            w = len(b_slots) * BS
            if not diag:
                e_t = ep.tile([128, RBC, 1024], FP8, name="e_t")
                em_t = emp.tile([128, RBC, 1024], FP8, name="em_t")
            for rb in range(RBC):
                ps = pp.tile([128, 1024], F32, name="ps")
                for h, b in enumerate(b_slots):
                    for t in range(KP):
                        nc.tensor.matmul(
                            ps[:, h * BS : (h + 1) * BS],
                            band_t[a_slot][:, 2 * t : 2 * t + 2, rb * 128 : (rb + 1) * 128],
                            band_t[b][:, 2 * t : 2 * t + 2, :],
                            start=(t == 0),
                            stop=(t == KP - 1),
                            perf_mode=DR,
                        )
                # Colsums of the PREVIOUS off-diag group: its e/em tiles are
                # ready by now, so the PE never waits on ACT/DVE, and the
                # copies/DMAs still clear well before the tail. For the final
                # (diag) group flush one rb later: its first exps then drain
                # during the flush, shortening the end-of-schedule ACT ladder.
                flush_rb = 1 if g == NG - 1 else 0
                if rb == flush_rb and pending_cols is not None:
                    pending_cols()
                    pending_cols = None
                slot = g * RBC + rb
                ycol = (0 if a_slot == 0 else 4) + rb
                yc_in = yc_t[:, b_slots[0] * BS : b_slots[0] * BS + w]
                if diag:
                    e_dg = dgp.tile([128, BS], BF16, name="e_dg")
                    nc.scalar.activation(
                        out=e_dg[:],
                        in_=ps[:, 0:BS],
                        func=mybir.ActivationFunctionType.Exp,
                        scale=ASCALE,
                        accum_out=stage[:, 3 * slot : 3 * slot + 1],
                    )
                    em_dg = dgp.tile([128, BS], BF16, name="em_dg")
                    nc.vector.scalar_tensor_tensor(
                        out=em_dg[:],
                        in0=yc_in,
                        scalar=yo_t[:, ycol : ycol + 1],
                        in1=e_dg[:],
                        op0=mybir.AluOpType.is_equal,
                        op1=mybir.AluOpType.mult,
                        accum_out=stage[:, 3 * slot + 2 : 3 * slot + 3],
                    )
                else:
                    nc.scalar.activation(
                        out=e_t[:, rb, 0:w],
                        in_=ps[:, 0:w],
                        func=mybir.ActivationFunctionType.Exp,
                        scale=ASCALE,
                        accum_out=stage[:, 3 * slot : 3 * slot + 1],
                    )
                    nc.vector.scalar_tensor_tensor(
                        out=em_t[:, rb, 0:w],
                        in0=yc_in,
                        scalar=yo_t[:, ycol : ycol + 1],
                        in1=e_t[:, rb, 0:w],
                        op0=mybir.AluOpType.is_equal,
                        op1=mybir.AluOpType.mult,
                        accum_out=stage[:, 3 * slot + 2 : 3 * slot + 3],
                    )

            if not diag:

                def make_cols(e_t=e_t, em_t=em_t, b_slots=b_slots, last=(g == NG - 2)):
                    def emit():
                        for h, b in enumerate(b_slots):
                            cs = csp.tile([1, 1024], F32, name="cs", tag="cs")
                            cps_e = cpp.tile([1, BS], F32, name="cps_e", tag="col")
                            for t2 in range(2):
                                nc.tensor.matmul(
                                    cps_e[:],
                                    ones_t[:, 0:2, 0:1],
                                    e_t[:, 2 * t2 : 2 * t2 + 2, h * BS : (h + 1) * BS],
                                    start=(t2 == 0),
                                    stop=(t2 == 1),
                                    perf_mode=DR,
                                )
                            cps_m = cpp.tile([1, BS], F32, name="cps_m", tag="col")
                            for t2 in range(2):
                                nc.tensor.matmul(
                                    cps_m[:],
                                    ones_t[:, 0:2, 0:1],
                                    em_t[:, 2 * t2 : 2 * t2 + 2, h * BS : (h + 1) * BS],
                                    start=(t2 == 0),
                                    stop=(t2 == 1),
                                    perf_mode=DR,
                                )
                            # Copies ride on the otherwise-idle GPSIMD so
                            # neither ACT (exp chain) nor DVE (mask chain)
                            # ever backs up behind them.
                            nc.gpsimd.tensor_copy(out=cs[:, 0:BS], in_=cps_e[:])
                            nc.gpsimd.tensor_copy(out=cs[:, BS:1024], in_=cps_m[:])
                            cidx = b - 1
                            col_dmas.append(
                                (cidx, cs)
                            )

                    return emit

                pending_cols = make_cols()

        if pending_cols is not None:
            pending_cols()
        # Tail: the bulk of rows h1 (groups 5..8) is ready before the final
        # diag finishes — issue it first so only the last 12 columns ride on
        # the final mask op. The last cols go out via the idle Pool queue so
        # the SP sequencer never blocks the rows chain.
        half = NG * RBC * 3 // 2
        tail = (NG - 1) * RBC * 3
        nc.sync.dma_start(out=rows_d[:, half:tail], in_=stage[:, half:tail])
        for cidx, cs in col_dmas:
            nc.sync.dma_start(out=cols_d[:, cidx * 1024 : (cidx + 1) * 1024], in_=cs[:])
        col_dmas.clear()
        nc.sync.dma_start(out=rows_d[:, tail:], in_=stage[:, tail:])

    nc.compile()
    return nc


_CACHE: dict = {}


def _get_nc():
    if "nc" not in _CACHE:
        _CACHE["nc"] = build_bass()
    return _CACHE["nc"]


def _prep_inputs(x, y):
    x = np.ascontiguousarray(np.asarray(x, dtype=np.float32))
    y = np.asarray(y).astype(np.int64)
    xn = x / np.linalg.norm(x, axis=1, keepdims=True)
    # Renormalize in the quantized domain: rescale each row so its QUANTIZED
    # norm is exactly QS, killing the systematic s_ii bias from fp8 rounding.
    xq = (xn * QS).astype(NP_FP8)
    nrm = np.sqrt((xq.astype(np.float32) ** 2).sum(1, keepdims=True)) / QS
    xq = ((xn * QS) / nrm).astype(NP_FP8)
    ybf = y.astype(NP_BF16)

    # band[b][p, kc, jj] = xq[b*BS + jj, kc*128 + p]
    bandmat = [
        np.ascontiguousarray(xq[b * BS : (b + 1) * BS].reshape(BS, KC, 128).transpose(2, 1, 0))
        for b in range(NB)
    ]

    in_maps = []
    for c in range(NCORES):
        perm = [(c + s) % NB for s in range(NB)]
        bands = np.concatenate([bandmat[b] for b in perm], axis=0)
        yc = np.ascontiguousarray(
            np.concatenate([ybf[b * BS : (b + 1) * BS] for b in perm])
        ).reshape(1, N)
        yo = np.ascontiguousarray(
            np.concatenate(
                [
                    ybf[perm[a] * BS : (perm[a] + 1) * BS].reshape(RBC, 128).T
                    for a in (0, 8)
                ],
                axis=1,
            )
        )
        in_maps.append({"bands": bands, "yc": yc, "yo": yo})
    return in_maps


def _assemble(results):
    sum_all = np.zeros(N, dtype=np.float64)
    sum_same = np.zeros(N, dtype=np.float64)
    for c in range(NCORES):
        r = results[c]
        perm = [(c + s) % NB for s in range(NB)]
        rows = r["rows"].astype(np.float64)        # [128, 120]: [all_h0, all_h1, same]
        cols = r["cols"].astype(np.float64).reshape(NOFF, 2, BS)
        for g, (a_slot, kind, b_slots) in enumerate(GROUPS):
            ab = perm[a_slot]
            for rb in range(RBC):
                rr = ab * BS + rb * 128 + np.arange(128)
                slot = g * RBC + rb
                sum_all[rr] += rows[:, 3 * slot] + rows[:, 3 * slot + 1]
                sum_same[rr] += rows[:, 3 * slot + 2]
            if kind != "diag":
                for b in b_slots:
                    cc = perm[b] * BS + np.arange(BS)
                    cidx = b - 1
                    sum_all[cc] += cols[cidx, 0]
                    sum_same[cc] += cols[cidx, 1]
    loss = np.log(sum_all) - np.log(sum_same)
    return np.float32(loss.mean())


def run(x, y, trace=False, **spmd_kwargs):
    nc = _get_nc()
    in_maps = _prep_inputs(x, y)
    res = run_bass_kernel_spmd(
        nc, in_maps, core_ids=list(range(NCORES)), trace=trace, **spmd_kwargs
    )
    return _assemble(res.results), res


def kernel(x, y, fp_v=None, **_ignored):
    val, _ = run(x, y, trace=False)
    return np.asarray(val, dtype=np.float32)
